# revision 4
# baseline (speedup 1.0000x reference)
"""Trainium2 Bass kernel for nn_Attention_51376398794919.

Dense transformer block: LayerNorm -> QKV -> attention with relative-position
bias -> proj.  Data-parallel over batch across 8 NeuronCores (4 batches/core).

Device-side layout strategy (per core):
  - LN in natural layout [tok, d]; xn transposed to xnT [d, tok] via PE
    transposes (stored bf16).
  - qkT (q/k head-transposed, [d_head, tok]) and v-natural ([tok, d_v])
    computed from xnT; q-scale and LN affine folded into weights on host.
  - Scores computed TRANSPOSED: ST[k, q] = kT.T @ qT (bias tiles are read
    [k, q]); the relative-position bias is added by the VECTOR engine
    (tensor_tensor add, in place on the score PSUM tile) instead of burning
    TensorE cycles on identity-matmul bias accumulation; exp on the scalar
    engine PSUM->SBUF (scores are provably < ~4 in magnitude so no
    max-subtraction is needed).
  - PV: out[q, d] = expST.T @ [v | ones | 0]; the ones column yields the
    softmax denominator for free; normalization is a per-partition
    tensor_scalar (258 pad keeps the free size even).
  - attn transposed back (PE) and kept in an SBUF slab; epilogue does proj.
All matmuls run in bf16 with fp32 PSUM accumulation.  PSUM->SBUF copies for
q/k/v run on the scalar engine (activation Copy) to keep the vector engine
free for the bias adds.  The head loop is software-pipelined (qkT/v of head
h+1 emitted between scores(h) and PV(h)).  K=1 bias matmuls are skipped when
the corresponding biases are all zero (checked on the host; build flags
select the general path otherwise).
"""

import sys

import numpy as np

sys.path.insert(0, "/opt/trn_rl_repo")

import concourse.bacc as bacc
import concourse.mybir as mybir
import concourse.tile as tile
from concourse.bass_utils import run_bass_kernel_spmd

# Problem constants
B, N, DIM = 32, 1024, 512
H, KD, D = 8, 64, 256
DH = D * H  # 2048
SCALE = KD ** -0.5
NCORES = 8
BL = B // NCORES  # 4 batches per core

F32 = mybir.dt.float32
BF16 = mybir.dt.bfloat16
AF = mybir.ActivationFunctionType
ALU = mybir.AluOpType

KT = N // 128    # 8 k-tiles
QS = N // 128    # 8 q-slices
DT = DIM // 128  # 4 d-tiles
VW = 258         # v-hat width: 256 v + 1 ones + 1 pad


def build_program(use_qk_bias=False, use_v_bias=False, use_pb=False):
    nc = bacc.Bacc("TRN2", target_bir_lowering=False, debug=True)

    x_d = nc.declare_dram_parameter("x", [BL, N, DIM], F32, isOutput=False)
    wqk_d = nc.declare_dram_parameter("wqk", [DIM, H * 128], BF16, isOutput=False)
    wv_d = nc.declare_dram_parameter("wv", [DIM, DH], BF16, isOutput=False)
    bqk_d = nc.declare_dram_parameter("bqk", [1, H * 128], BF16, isOutput=False)
    bv1_d = nc.declare_dram_parameter("bv1", [1, H * VW], BF16, isOutput=False)
    pw_d = nc.declare_dram_parameter("pw", [DH, DIM], BF16, isOutput=False)
    pb1_d = nc.declare_dram_parameter("pb1", [1, DIM], BF16, isOutput=False)
    bias_d = nc.declare_dram_parameter("bias", [H, N, N], BF16, isOutput=False)
    identb_d = nc.declare_dram_parameter("identb", [128, 128], BF16, isOutput=False)
    ones_d = nc.declare_dram_parameter("ones", [1, 512], BF16, isOutput=False)
    y_d = nc.declare_dram_parameter("y", [BL, N, DIM], F32, isOutput=True)

    any_bias = use_qk_bias or use_v_bias or use_pb

    with tile.TileContext(nc) as tc:
        with (
            tc.tile_pool(name="consts", bufs=1) as cpool,
            tc.tile_pool(name="xnt", bufs=2) as xpool,
            tc.tile_pool(name="slab", bufs=1) as slabpool,
            tc.tile_pool(name="yout", bufs=3) as ypool,
            tc.tile_pool(name="ln", bufs=4) as lpool,
            tc.tile_pool(name="stats", bufs=8) as spool,
            tc.tile_pool(name="bias", bufs=12) as bpool,
            tc.tile_pool(name="qk", bufs=3) as qkpool,
            tc.tile_pool(name="vhat", bufs=3) as vpool,
            tc.tile_pool(name="expst", bufs=18) as epool,
            tc.tile_pool(name="attn", bufs=8) as apool,
            tc.tile_pool(name="stp", bufs=3, space="PSUM") as stpp,
            tc.tile_pool(name="pvp", bufs=3, space="PSUM") as pvpp,
            tc.tile_pool(name="miscp", bufs=2, space="PSUM") as mpp,
        ):
            # ---- constants ----
            if any_bias:
                ones_bf = cpool.tile([1, 512], BF16)
                nc.sync.dma_start(ones_bf[:], ones_d[:])
            identb = cpool.tile([128, 128], BF16)
            nc.sync.dma_start(identb[:], identb_d[:])
            eps_t = cpool.tile([128, 1], F32)
            nc.vector.memset(eps_t[:], 1e-5)
            zero_t = cpool.tile([128, 1], F32)
            nc.vector.memset(zero_t[:], 0.0)
            if use_qk_bias:
                bqk = cpool.tile([1, H * 128], BF16)
                nc.sync.dma_start(bqk[:], bqk_d[:])
            if use_v_bias:
                bv1 = cpool.tile([1, H * VW], BF16)
                nc.sync.dma_start(bv1[:], bv1_d[:])
            wqk = cpool.tile([128, DT * H * 128], BF16)  # [d-tile][dpart, f]
            for dt in range(DT):
                for hh in range(2):
                    nc.sync.dma_start(
                        wqk[:, dt * H * 128 + hh * 512:
                            dt * H * 128 + (hh + 1) * 512],
                        wqk_d[dt * 128:(dt + 1) * 128,
                              hh * 512:(hh + 1) * 512],
                    )
            wv = cpool.tile([128, DT * DH], BF16)
            for dt in range(DT):
                for hh in range(2):
                    nc.sync.dma_start(
                        wv[:, dt * DH + hh * (DH // 2):
                           dt * DH + (hh + 1) * (DH // 2)],
                        wv_d[dt * 128:(dt + 1) * 128,
                             hh * (DH // 2):(hh + 1) * (DH // 2)],
                    )
            if use_pb:
                pb1 = cpool.tile([1, DIM], BF16)
                nc.sync.dma_start(pb1[:], pb1_d[:])
            pw = cpool.tile([128, 16 * DIM], BF16)
            for dh in range(16):
                nc.sync.dma_start(
                    pw[:, dh * DIM:(dh + 1) * DIM],
                    pw_d[dh * 128:(dh + 1) * 128, :],
                )

            # ---- batch-outer main loop ----
            for b in range(BL):
                # LN + transpose -> xnT (bf16)
                xnt = xpool.tile([128, DT * N], BF16, tag="xnt")
                for sl in range(QS):  # tok-slices of 128
                    xt = lpool.tile([128, DIM], F32, tag="x")
                    nc.sync.dma_start(xt[:], x_d[b, sl * 128:(sl + 1) * 128, :])
                    st6 = spool.tile([128, 6], F32, tag="st6")
                    nc.vector.bn_stats(st6[:], xt[:])
                    mv = spool.tile([128, 2], F32, tag="mv")
                    nc.vector.bn_aggr(mv[:], st6[:])
                    sd = spool.tile([128, 1], F32, tag="sd")
                    nc.scalar.activation(sd[:], mv[:, 1:2], AF.Sqrt, bias=eps_t[:])
                    rs = spool.tile([128, 1], F32, tag="rs")
                    nc.vector.reciprocal(rs[:], sd[:])
                    nm = spool.tile([128, 1], F32, tag="nm")
                    nc.vector.tensor_scalar(
                        nm[:], mv[:, 0:1], rs[:], -1.0, ALU.mult, ALU.mult
                    )
                    xn = lpool.tile([128, DIM], BF16, tag="xn")
                    nc.vector.tensor_scalar(
                        xn[:], xt[:], rs[:], nm[:], ALU.mult, ALU.add
                    )
                    for dt in range(DT):
                        tp = mpp.tile([128, 128], BF16, tag="m")
                        nc.tensor.transpose(
                            tp[:], xn[:, dt * 128:(dt + 1) * 128], identb[:]
                        )
                        nc.vector.tensor_copy(
                            xnt[:, dt * N + sl * 128: dt * N + (sl + 1) * 128],
                            tp[:],
                        )

                slab = slabpool.tile([128, 16 * N], BF16, tag="slab")

                def emit_bias_dma(h):
                    btiles = []
                    for kt in range(KT):
                        bt = bpool.tile([128, N], BF16, tag="bias")
                        nc.sync.dma_start(
                            bt[:], bias_d[h, kt * 128:(kt + 1) * 128, :]
                        )
                        btiles.append(bt)
                    return btiles

                def alloc_qkv(h):
                    # qp rows 0:64 = q (SCALE folded on host), rows 64:128 = k
                    qt = qkpool.tile([64, N], BF16, tag="qt")
                    ktile = qkpool.tile([64, N], BF16, tag="kt")
                    vh = vpool.tile([128, KT * VW], BF16, tag="vh")
                    if not use_v_bias:
                        nc.vector.memset(
                            vh[:].rearrange("p (s w) -> p s w", w=VW)[:, :, 256:258],
                            0.0,
                        )
                        nc.vector.memset(
                            vh[:].rearrange("p (s w) -> p s w", w=VW)[:, :, 256:257],
                            1.0,
                        )
                    return qt, ktile, vh

                def emit_qk_group(h, qkv, c):
                    qt, ktile, vh = qkv
                    qp = mpp.tile([128, 512], F32, tag="m")
                    for dt in range(DT):
                        nc.tensor.matmul(
                            qp[:],
                            wqk[:, dt * H * 128 + h * 128:
                                dt * H * 128 + (h + 1) * 128],
                            xnt[:, dt * N + c * 512: dt * N + (c + 1) * 512],
                            start=(dt == 0),
                            stop=(not use_qk_bias and dt == DT - 1),
                        )
                    if use_qk_bias:
                        nc.tensor.matmul(
                            qp[:],
                            bqk[:, h * 128:(h + 1) * 128],
                            ones_bf[:, 0:512],
                            start=False,
                            stop=True,
                        )
                    nc.scalar.activation(
                        qt[:, c * 512:(c + 1) * 512], qp[0:64, :], AF.Copy
                    )
                    nc.scalar.activation(
                        ktile[:, c * 512:(c + 1) * 512], qp[64:128, :], AF.Copy
                    )

                def emit_v_group(h, qkv, sl):
                    qt, ktile, vh = qkv
                    vp = mpp.tile([128, 512], F32, tag="m")
                    for dt in range(DT):
                        nc.tensor.matmul(
                            vp[:, 0:256],
                            xnt[:, dt * N + sl * 128: dt * N + (sl + 1) * 128],
                            wv[:, dt * DH + h * 256: dt * DH + (h + 1) * 256],
                            start=(dt == 0),
                            stop=(not use_v_bias and dt == DT - 1),
                        )
                    if use_v_bias:
                        nc.tensor.matmul(
                            vp[:, 0:VW],
                            ones_bf[:, 0:128],
                            bv1[:, h * VW:(h + 1) * VW],
                            start=False,
                            stop=True,
                            skip_group_check=True,
                        )
                        nc.scalar.activation(
                            vh[:, sl * VW:(sl + 1) * VW], vp[:, 0:VW], AF.Copy
                        )
                    else:
                        nc.scalar.activation(
                            vh[:, sl * VW: sl * VW + 256], vp[:, 0:256], AF.Copy
                        )

                def emit_score(hctx, et, kt, c):
                    btiles, (qt, ktile, vh) = hctx
                    cs = slice(c * 512, (c + 1) * 512)
                    sp = stpp.tile([128, 512], F32, tag="st")
                    nc.tensor.matmul(
                        sp[:],
                        ktile[:, kt * 128:(kt + 1) * 128],
                        qt[:, cs],
                        start=True, stop=True,
                    )
                    # bias add on the vector engine, in place in PSUM
                    nc.vector.tensor_tensor(
                        sp[:], sp[:], btiles[kt][:, cs], ALU.add
                    )
                    nc.scalar.activation(et[:, cs], sp[:], AF.Exp,
                                         bias=zero_t[:])

                def emit_pv_slice(h, hctx, est, sl):
                    btiles, (qt, ktile, vh) = hctx
                    pv = pvpp.tile([128, VW], F32, tag="pv")
                    for kt in range(KT):
                        nc.tensor.matmul(
                            pv[:],
                            est[kt][:, sl * 128:(sl + 1) * 128],
                            vh[:, kt * VW:(kt + 1) * VW],
                            start=(kt == 0),
                            stop=(kt == KT - 1),
                        )
                    rc = spool.tile([128, 1], F32, tag="rc")
                    nc.vector.reciprocal(rc[:], pv[:, 256:257])
                    an = apool.tile([128, 256], BF16, tag="an")
                    nc.vector.tensor_scalar(
                        an[:], pv[:, 0:256], rc[:], None, ALU.mult
                    )
                    for dt in range(2):
                        tp = mpp.tile([128, 128], BF16, tag="m")
                        nc.tensor.transpose(
                            tp[:], an[:, dt * 128:(dt + 1) * 128], identb[:]
                        )
                        nc.vector.tensor_copy(
                            slab[:, (h * 2 + dt) * N + sl * 128:
                                 (h * 2 + dt) * N + (sl + 1) * 128],
                            tp[:],
                        )

                def emit_head(h, hctx, nxt_qkv):
                    """Scores+exp of head h, interleaved with qkv of h+1 (to
                    keep PE fed while exp drains the score PSUM tiles), then
                    PV of head h."""
                    # next head's qkv work, sliced into 10 matmul groups
                    pend = []
                    if nxt_qkv is not None:
                        pend = [lambda c=c: emit_qk_group(h + 1, nxt_qkv, c)
                                for c in range(2)]
                        pend += [lambda sl=sl: emit_v_group(h + 1, nxt_qkv, sl)
                                 for sl in range(QS)]
                    est = []
                    gi = 0
                    for kt in range(KT):
                        et = epool.tile([128, N], BF16, tag="e")
                        for c in range(2):
                            emit_score(hctx, et, kt, c)
                            # 10 qkv groups spread over the 16 score tiles
                            take = 1 if (kt * 2 + c) % 8 < 5 else 0
                            for _ in range(take):
                                if gi < len(pend):
                                    pend[gi]()
                                    gi += 1
                        est.append(et)
                    while gi < len(pend):
                        pend[gi]()
                        gi += 1
                    for sl in range(QS):
                        emit_pv_slice(h, hctx, est, sl)

                # software-pipelined head loop: bias DMA + qkv one head ahead
                btiles0 = emit_bias_dma(0)
                qkv0 = alloc_qkv(0)
                for c in range(2):
                    emit_qk_group(0, qkv0, c)
                for sl in range(QS):
                    emit_v_group(0, qkv0, sl)
                hctx = (btiles0, qkv0)
                for h in range(H):
                    if h + 1 < H:
                        nb = emit_bias_dma(h + 1)
                        nq = alloc_qkv(h + 1)
                    else:
                        nb, nq = None, None
                    emit_head(h, hctx, nq)
                    hctx = (nb, nq) if nq is not None else None

                # proj for batch b from the SBUF slab
                for sl in range(QS):
                    pp = stpp.tile([128, 512], F32, tag="st")
                    for dh in range(16):
                        nc.tensor.matmul(
                            pp[:],
                            slab[:, dh * N + sl * 128: dh * N + (sl + 1) * 128],
                            pw[:, dh * DIM:(dh + 1) * DIM],
                            start=(dh == 0),
                            stop=(not use_pb and dh == 15),
                        )
                    if use_pb:
                        nc.tensor.matmul(
                            pp[:], ones_bf[:, 0:128], pb1[:], start=False,
                            stop=True, skip_group_check=True,
                        )
                    yt = ypool.tile([128, DIM], F32, tag="y")
                    nc.scalar.activation(yt[:], pp[:], AF.Copy)
                    nc.sync.dma_start(y_d[b, sl * 128:(sl + 1) * 128, :], yt[:])

    nc.compile()
    return nc


_CACHE = {}


def _prep_host(gamma, beta, qkv_w, qkv_b, proj_w, proj_b, biases, bias_idxs):
    import ml_dtypes

    qkv_w = np.asarray(qkv_w, np.float32)
    qkv_b = np.asarray(qkv_b, np.float32)
    gamma = np.asarray(gamma, np.float32)
    beta = np.asarray(beta, np.float32)
    w = qkv_w * gamma[:, None]          # fold LN gamma
    bfold = qkv_b + beta @ qkv_w        # fold LN beta
    w3 = w.reshape(DIM, H, 384)
    b3 = bfold.reshape(H, 384)
    # q/k columns, q scaled by SCALE
    wqk = np.concatenate([w3[:, :, :64] * SCALE, w3[:, :, 64:128]], axis=2)
    wqk = wqk.reshape(DIM, H * 128)
    bqk = np.concatenate([b3[:, :64] * SCALE, b3[:, 64:128]], axis=1)
    bqk = bqk.reshape(1, H * 128)
    wv = w3[:, :, 128:].reshape(DIM, DH)
    bv = b3[:, 128:]                    # [H, 256]
    bv1 = np.concatenate(
        [bv, np.ones((H, 1), np.float32), np.zeros((H, 1), np.float32)],
        axis=1,
    ).reshape(1, H * VW)
    bias_full = np.asarray(biases, np.float32)[:, np.asarray(bias_idxs)]
    # device reads bias tiles as [k, q]; transpose (a no-op for the
    # symmetric |dr|,|dc| relative-position bias, but correct in general)
    bias_full = bias_full.transpose(0, 2, 1)
    return {
        "wqk": wqk.astype(ml_dtypes.bfloat16),
        "wv": wv.astype(ml_dtypes.bfloat16),
        "bqk": bqk.astype(ml_dtypes.bfloat16),
        "bv1": bv1.astype(ml_dtypes.bfloat16),
        "pw": np.ascontiguousarray(np.asarray(proj_w, np.float32)).astype(ml_dtypes.bfloat16),
        "pb1": np.asarray(proj_b, np.float32).reshape(1, DIM).astype(ml_dtypes.bfloat16),
        "bias": np.ascontiguousarray(bias_full).astype(ml_dtypes.bfloat16),
        "identb": np.eye(128, dtype=np.float32).astype(ml_dtypes.bfloat16),
        "ones": np.ones((1, 512), ml_dtypes.bfloat16),
    }


def kernel(x, gamma, beta, qkv_w, qkv_b, proj_w, proj_b, biases, bias_idxs,
           _trace=False, _tmpdir=None):
    x = np.asarray(x, np.float32)
    shared = _prep_host(gamma, beta, qkv_w, qkv_b, proj_w, proj_b, biases,
                        bias_idxs)
    flags = (
        bool(np.any(np.asarray(shared["bqk"], np.float32))),
        bool(np.any(np.asarray(shared["bv1"], np.float32)
                    .reshape(H, VW)[:, :256])),
        bool(np.any(np.asarray(shared["pb1"], np.float32))),
    )
    if _CACHE.get("flags") != flags:
        _CACHE["nc"] = build_program(*flags)
        _CACHE["flags"] = flags
    nc = _CACHE["nc"]
    in_maps = []
    for c in range(NCORES):
        m = dict(shared)
        m["x"] = np.ascontiguousarray(x[c * BL:(c + 1) * BL])
        in_maps.append(m)
    res = run_bass_kernel_spmd(
        nc, in_maps, list(range(NCORES)), trace=_trace, tmpdir=_tmpdir,
    )
    _CACHE["last"] = res
    out = np.concatenate([res.results[c]["y"] for c in range(NCORES)], axis=0)
    return out.astype(np.float32)


# revision 26
# speedup vs baseline: 1.1947x; 1.1947x over previous
"""Trainium2 Bass kernel for nn_Attention_51376398794919.

Dense transformer block: LayerNorm -> QKV -> attention with relative-position
bias -> proj.  Data-parallel over batch across 8 NeuronCores (4 batches/core).

Device-side layout strategy (per core):
  - LN in natural layout [tok, d]; xn transposed to xnT [d, tok] via PE
    transposes (stored bf16).
  - qkT (q/k head-transposed, [d_head, tok]) and v-natural ([tok, d_v])
    computed from xnT; q-scale and LN affine folded into weights on host.
  - Scores computed TRANSPOSED: ST[k, q] = kT.T @ qT (bias tiles are read
    [k, q]); the relative-position bias is added by the VECTOR engine
    (tensor_tensor add, in place on the score PSUM tile) instead of burning
    TensorE cycles on identity-matmul bias accumulation; exp on the scalar
    engine PSUM->SBUF (scores are provably < ~4 in magnitude so no
    max-subtraction is needed).
  - PV: out[q, d] = expST.T @ [v | ones | 0]; the ones column yields the
    softmax denominator for free; normalization is a per-partition
    tensor_scalar (258 pad keeps the free size even).
  - attn transposed back (PE) and kept in an SBUF slab; epilogue does proj.
All matmuls run in bf16 with fp32 PSUM accumulation.  PSUM->SBUF copies for
q/k/v run on the scalar engine (activation Copy) to keep the vector engine
free for the bias adds.  The head loop is software-pipelined (qkT/v of head
h+1 emitted between scores(h) and PV(h)).  K=1 bias matmuls are skipped when
the corresponding biases are all zero (checked on the host; build flags
select the general path otherwise).
"""

import sys

import numpy as np

sys.path.insert(0, "/opt/trn_rl_repo")

import concourse.bacc as bacc
import concourse.mybir as mybir
import concourse.tile as tile
from concourse.ap import AP
from concourse.bass_utils import run_bass_kernel_spmd


def strided2(ap, start, stride, n):
    """[128, 2, n] view of a 2D tile AP: two n-wide column blocks at
    `start` and `start+stride` (free-dim elements)."""
    return AP(ap.tensor, ap.offset + start,
              [list(ap.ap[0]), [stride, 2], [1, n]])

# Problem constants
B, N, DIM = 32, 1024, 512
H, KD, D = 8, 64, 256
DH = D * H  # 2048
SCALE = KD ** -0.5
NCORES = 8
BL = B // NCORES  # 4 batches per core

F32 = mybir.dt.float32
BF16 = mybir.dt.bfloat16
AF = mybir.ActivationFunctionType
ALU = mybir.AluOpType

KT = N // 128    # 8 k-tiles
QS = N // 128    # 8 q-slices
DT = DIM // 128  # 4 d-tiles
VW = 258         # v-hat width: 256 v + 1 ones + 1 pad


def build_program(use_qk_bias=False, use_v_bias=False, use_pb=False):
    nc = bacc.Bacc("TRN2", target_bir_lowering=False, debug=True)

    x_d = nc.declare_dram_parameter("x", [BL, N, DIM], F32, isOutput=False)
    wqk_d = nc.declare_dram_parameter("wqk", [DIM, H * 128], BF16, isOutput=False)
    wv_d = nc.declare_dram_parameter("wv", [DIM, DH], BF16, isOutput=False)
    bqk_d = nc.declare_dram_parameter("bqk", [1, H * 128], BF16, isOutput=False)
    bv1_d = nc.declare_dram_parameter("bv1", [1, H * VW], BF16, isOutput=False)
    pw_d = nc.declare_dram_parameter("pw", [DH, DIM], BF16, isOutput=False)
    pb1_d = nc.declare_dram_parameter("pb1", [1, DIM], BF16, isOutput=False)
    bias_d = nc.declare_dram_parameter("bias", [H, N, N], BF16, isOutput=False)
    identb_d = nc.declare_dram_parameter("identb", [128, 128], BF16, isOutput=False)
    ones_d = nc.declare_dram_parameter("ones", [1, 512], BF16, isOutput=False)
    y_d = nc.declare_dram_parameter("y", [BL, N, DIM], F32, isOutput=True)

    any_bias = use_qk_bias or use_v_bias or use_pb

    with tile.TileContext(nc) as tc:
        with (
            tc.tile_pool(name="consts", bufs=1) as cpool,
            tc.tile_pool(name="xnt", bufs=2) as xpool,
            tc.tile_pool(name="slab", bufs=1) as slabpool,
            tc.tile_pool(name="yout", bufs=2) as ypool,
            tc.tile_pool(name="ln", bufs=3) as lpool,
            tc.tile_pool(name="xin", bufs=1) as xinpool,
            tc.tile_pool(name="stats", bufs=8) as spool,
            tc.tile_pool(name="bias", bufs=4) as bpool,
            tc.tile_pool(name="qk", bufs=3) as qkpool,
            tc.tile_pool(name="vhat", bufs=4) as vpool,
            tc.tile_pool(name="expst", bufs=16) as epool,
            tc.tile_pool(name="attn", bufs=4) as apool,
            tc.tile_pool(name="stp", bufs=2, space="PSUM") as stpp,
            tc.tile_pool(name="pvp", bufs=2, space="PSUM") as pvpp,
            tc.tile_pool(name="miscp", bufs=2, space="PSUM") as mpp,
        ):
            # ---- constants ----
            if any_bias:
                ones_bf = cpool.tile([1, 512], BF16)
                nc.sync.dma_start(ones_bf[:], ones_d[:])
            identb = cpool.tile([128, 128], BF16)
            nc.sync.dma_start(identb[:], identb_d[:])
            eps_t = cpool.tile([128, 1], F32)
            nc.vector.memset(eps_t[:], 1e-5)
            zero_t = cpool.tile([128, 1], F32)
            nc.vector.memset(zero_t[:], 0.0)

            def emit_x_load(b):
                # whole batch of x in two big DMAs (keeps the DMA engines
                # from interleaving 8 small x loads behind weight/bias
                # transfers at startup)
                xf = xinpool.tile([128, QS, DIM], F32, tag="xf")
                src3 = x_d[b].rearrange("(a p) q -> p a q", p=128)
                for g in range(2):
                    nc.gpsimd.dma_start(xf[:, g * 4:(g + 1) * 4, :],
                                        src3[:, g * 4:(g + 1) * 4, :])
                return xf

            xf_cur = emit_x_load(0)
            if use_qk_bias:
                bqk = cpool.tile([1, H * 128], BF16)
                nc.sync.dma_start(bqk[:], bqk_d[:])
            if use_v_bias:
                bv1 = cpool.tile([1, H * VW], BF16)
                nc.sync.dma_start(bv1[:], bv1_d[:])
            # weights: single rearranged DMAs; SP-queue order after identb
            # gives the DMA-engine arrival order x -> identb -> wqk -> wv ->
            # bias(h0); pw is deferred to the pool queue (needed only by the
            # proj epilogue)
            wqk = cpool.tile([128, DT * H * 128], BF16)  # [d-tile][dpart, f]
            nc.sync.dma_start(
                wqk[:].rearrange("p (a q) -> p a q", a=DT),
                wqk_d[:].rearrange("(a p) q -> p a q", p=128),
            )
            wv = cpool.tile([128, DT * DH], BF16)
            nc.sync.dma_start(
                wv[:].rearrange("p (a q) -> p a q", a=DT),
                wv_d[:].rearrange("(a p) q -> p a q", p=128),
            )
            if use_pb:
                pb1 = cpool.tile([1, DIM], BF16)
                nc.sync.dma_start(pb1[:], pb1_d[:])
            pw = cpool.tile([128, 16 * DIM], BF16)

            # ---- emitters (batch-parametrized) ----
            def emit_ln_slice(xf, xnt, sl):
                xt = xf[:, sl, :]
                st6 = spool.tile([128, 6], F32, tag="st6")
                nc.vector.bn_stats(st6[:], xt[:])
                mv = spool.tile([128, 2], F32, tag="mv")
                nc.vector.bn_aggr(mv[:], st6[:])
                sd = spool.tile([128, 1], F32, tag="sd")
                nc.scalar.activation(sd[:], mv[:, 1:2], AF.Sqrt, bias=eps_t[:])
                rs = spool.tile([128, 1], F32, tag="rs")
                nc.vector.reciprocal(rs[:], sd[:])
                nm = spool.tile([128, 1], F32, tag="nm")
                nc.vector.tensor_scalar(
                    nm[:], mv[:, 0:1], rs[:], -1.0, ALU.mult, ALU.mult
                )
                xn = lpool.tile([128, DIM], BF16, tag="xn")
                nc.vector.tensor_scalar(
                    xn[:], xt, rs[:], nm[:], ALU.mult, ALU.add
                )
                # 4 transposes into one PSUM tile, one copy out
                tp = mpp.tile([128, 512], BF16, tag="m")
                for dt in range(DT):
                    nc.tensor.transpose(
                        tp[:, dt * 128:(dt + 1) * 128],
                        xn[:, dt * 128:(dt + 1) * 128], identb[:]
                    )
                nc.vector.tensor_copy(
                    xnt[:].rearrange("p (d n) -> p d n", n=N)[:, :, sl * 128:
                                                             (sl + 1) * 128],
                    tp[:].rearrange("p (d n) -> p d n", n=128),
                )

            def emit_bias_dma(h):
                # two [128, 4, 1024] tiles per head (4 k-tiles each),
                # issued from the (otherwise idle) gpsimd SWDGE queue
                src = bias_d[h].rearrange("(a p) q -> p a q", p=128)
                btiles = []
                for g in range(2):
                    bt = bpool.tile([128, 4, N], BF16, tag="bias")
                    nc.sync.dma_start(bt[:], src[:, g * 4:(g + 1) * 4, :])
                    btiles.append(bt)
                return btiles

            def alloc_qkv(h):
                # qp rows 0:64 = q (SCALE folded on host), rows 64:128 = k
                qt = qkpool.tile([64, N], BF16, tag="qt")
                ktile = qkpool.tile([64, N], BF16, tag="kt")
                vh = vpool.tile([128, KT * VW], BF16, tag="vh")
                nc.vector.memset(
                    vh[:].rearrange("p (s w) -> p s w", w=VW)[:, :, 256:258],
                    0.0,
                )
                nc.vector.memset(
                    vh[:].rearrange("p (s w) -> p s w", w=VW)[:, :, 256:257],
                    1.0,
                )
                return qt, ktile, vh

            def emit_qk_group(h, qkv, c, xnt):
                qt, ktile, vh = qkv
                qp = mpp.tile([128, 512], F32, tag="m")
                for dt in range(DT):
                    nc.tensor.matmul(
                        qp[:],
                        wqk[:, dt * H * 128 + h * 128:
                            dt * H * 128 + (h + 1) * 128],
                        xnt[:, dt * N + c * 512: dt * N + (c + 1) * 512],
                        start=(dt == 0),
                        stop=(not use_qk_bias and dt == DT - 1),
                    )
                if use_qk_bias:
                    nc.tensor.matmul(
                        qp[:],
                        bqk[:, h * 128:(h + 1) * 128],
                        ones_bf[:, 0:512],
                        start=False,
                        stop=True,
                    )
                nc.scalar.activation(
                    qt[:, c * 512:(c + 1) * 512], qp[0:64, :], AF.Copy
                )
                nc.scalar.activation(
                    ktile[:, c * 512:(c + 1) * 512], qp[64:128, :], AF.Copy
                )

            def emit_v_group(h, qkv, slp, xnt):
                # two tok-slices (2*slp, 2*slp+1) share one PSUM tile and
                # one PSUM->SBUF copy (strided dst into the vh layout)
                qt, ktile, vh = qkv
                vp = mpp.tile([128, 512], F32, tag="m")
                for i in range(2):
                    sl = 2 * slp + i
                    for dt in range(DT):
                        nc.tensor.matmul(
                            vp[:, i * 256:(i + 1) * 256],
                            xnt[:, dt * N + sl * 128: dt * N + (sl + 1) * 128],
                            wv[:, dt * DH + h * 256: dt * DH + (h + 1) * 256],
                            start=(dt == 0),
                            stop=(not use_v_bias and dt == DT - 1),
                        )
                    if use_v_bias:
                        nc.tensor.matmul(
                            vp[:, i * 256:(i + 1) * 256],
                            ones_bf[:, 0:128],
                            bv1[:, h * VW:h * VW + 256],
                            start=False,
                            stop=True,
                            skip_group_check=True,
                        )
                nc.scalar.activation(
                    vh[:].rearrange("p (s w) -> p s w", w=VW)[
                        :, 2 * slp:2 * slp + 2, 0:256],
                    vp[:].rearrange("p (a w) -> p a w", w=256),
                    AF.Copy,
                )

            def emit_score(hctx, et, kt):
                # one [128, 1024] PSUM tile (2 banks): 2 matmuls, then a
                # single bias add (vector) + exp (scalar) over 1024 cols
                btiles, (qt, ktile, vh) = hctx
                sp = stpp.tile([128, 1024], F32, tag="st")
                for c in range(2):
                    nc.tensor.matmul(
                        sp[:, c * 512:(c + 1) * 512],
                        ktile[:, kt * 128:(kt + 1) * 128],
                        qt[:, c * 512:(c + 1) * 512],
                        start=True, stop=True,
                    )
                nc.vector.tensor_tensor(
                    sp[:], sp[:], btiles[kt // 4][:, kt % 4, :], ALU.add
                )
                nc.scalar.activation(et[:], sp[:], AF.Exp, bias=zero_t[:])

            def emit_pv_slice(h, hctx, est, sl, slab):
                btiles, (qt, ktile, vh) = hctx
                pv = pvpp.tile([128, VW], F32, tag="pv")
                for kt in range(KT):
                    nc.tensor.matmul(
                        pv[:],
                        est[kt][:, sl * 128:(sl + 1) * 128],
                        vh[:, kt * VW:(kt + 1) * VW],
                        start=(kt == 0),
                        stop=(kt == KT - 1),
                    )
                rc = spool.tile([128, 1], F32, tag="rc")
                nc.vector.reciprocal(rc[:], pv[:, 256:257])
                an = apool.tile([128, 256], BF16, tag="an")
                if sl % 2 == 0:
                    # normalize on the scalar engine (Copy with AP scale)
                    nc.scalar.activation(an[:], pv[:, 0:256], AF.Copy,
                                         scale=rc[:])
                else:
                    nc.vector.tensor_scalar(
                        an[:], pv[:, 0:256], rc[:], None, ALU.mult
                    )
                tp = mpp.tile([128, 256], BF16, tag="m")
                for dt in range(2):
                    nc.tensor.transpose(
                        tp[:, dt * 128:(dt + 1) * 128],
                        an[:, dt * 128:(dt + 1) * 128], identb[:]
                    )
                nc.vector.tensor_copy(
                    slab[:].rearrange("p (d n) -> p d n", n=N)[
                        :, h * 2:h * 2 + 2, sl * 128:(sl + 1) * 128],
                    tp[:].rearrange("p (a w) -> p a w", w=128),
                )

            def emit_proj_slice(b, slab, sl):
                pp = stpp.tile([128, 512], F32, tag="st")
                for dh in range(16):
                    nc.tensor.matmul(
                        pp[:],
                        slab[:, dh * N + sl * 128: dh * N + (sl + 1) * 128],
                        pw[:, dh * DIM:(dh + 1) * DIM],
                        start=(dh == 0),
                        stop=(not use_pb and dh == 15),
                    )
                if use_pb:
                    nc.tensor.matmul(
                        pp[:], ones_bf[:, 0:128], pb1[:], start=False,
                        stop=True, skip_group_check=True,
                    )
                yt = ypool.tile([128, DIM], F32, tag="y")
                nc.scalar.activation(yt[:], pp[:], AF.Copy)
                nc.gpsimd.dma_start(y_d[b, sl * 128:(sl + 1) * 128, :], yt[:])

            def emit_phase(h, hctx, nxt_qkv, prev, xnt, slab, pre=()):
                """Phase h: score groups of head h interleaved with PV
                slices of head h-1 and qkv matmuls of head h+1, so PE
                always has ready work while the vector/scalar engines
                drain the score tiles (bias add + exp)."""
                pend = list(pre)
                if nxt_qkv is not None:
                    pend += [lambda c=c: emit_qk_group(h + 1, nxt_qkv, c, xnt)
                             for c in range(2)]
                    pend += [lambda p=p: emit_v_group(h + 1, nxt_qkv, p, xnt)
                             for p in range(QS // 2)]
                est = []
                gi = 0
                for kt in range(KT):
                    et = epool.tile([128, N], BF16, tag="e")
                    emit_score(hctx, et, kt)
                    if prev is not None:
                        emit_pv_slice(h - 1, prev[0], prev[1], kt, slab)
                    if kt % 4 != 3 and gi < len(pend):
                        pend[gi]()
                        gi += 1
                    est.append(et)
                while gi < len(pend):
                    pend[gi]()
                    gi += 1
                return est

            # ---- software-pipelined batch loop ----
            # batch 0 prologue: LN, bias DMA, qkv of head 0
            xnt_cur = xpool.tile([128, DT * N], BF16, tag="xnt")
            for sl in range(QS):
                emit_ln_slice(xf_cur, xnt_cur, sl)
            slab_cur = slabpool.tile([128, 16 * N], BF16, tag="slab")
            btiles0 = emit_bias_dma(0)
            qkv0 = alloc_qkv(0)
            for c in range(2):
                emit_qk_group(0, qkv0, c, xnt_cur)
            nc.gpsimd.dma_start(
                pw[:].rearrange("p (a q) -> p a q", a=16),
                pw_d[:].rearrange("(a p) q -> p a q", p=128),
            )
            hctx = (btiles0, qkv0)
            vpre = [lambda p=p: emit_v_group(0, qkv0, p, xnt_cur)
                    for p in range(QS // 2)]

            for b in range(BL):
                prev = None
                for h in range(H):
                    if h + 1 < H:
                        nb = emit_bias_dma(h + 1)
                        nq = alloc_qkv(h + 1)
                    else:
                        nb, nq = None, None
                    est = emit_phase(h, hctx, nq, prev, xnt_cur, slab_cur,
                                     pre=vpre)
                    vpre = ()
                    prev = (hctx, est)
                    hctx = (nb, nq) if nq is not None else None
                # fused tail: drain PV of head 7 + proj(b) + LN(b+1),
                # interleaved to keep all engines fed across the batch edge
                if b + 1 < BL:
                    xnt_next = xpool.tile([128, DT * N], BF16, tag="xnt")
                    xf_cur = emit_x_load(b + 1)
                else:
                    xnt_next = None
                for sl in range(QS):
                    emit_pv_slice(H - 1, prev[0], prev[1], sl, slab_cur)
                    if sl > 0:
                        emit_proj_slice(b, slab_cur, sl - 1)
                    if xnt_next is not None:
                        emit_ln_slice(xf_cur, xnt_next, sl)
                emit_proj_slice(b, slab_cur, QS - 1)
                if b + 1 < BL:
                    xnt_cur = xnt_next
                    slab_cur = slabpool.tile([128, 16 * N], BF16, tag="slab")
                    btiles0 = emit_bias_dma(0)
                    qkv0 = alloc_qkv(0)
                    for c in range(2):
                        emit_qk_group(0, qkv0, c, xnt_cur)
                    hctx = (btiles0, qkv0)
                    vpre = [lambda p=p, q=qkv0, x=xnt_cur:
                            emit_v_group(0, q, p, x)
                            for p in range(QS // 2)]

    nc.compile()
    return nc


_CACHE = {}


def _prep_host(gamma, beta, qkv_w, qkv_b, proj_w, proj_b, biases, bias_idxs):
    import ml_dtypes

    qkv_w = np.asarray(qkv_w, np.float32)
    qkv_b = np.asarray(qkv_b, np.float32)
    gamma = np.asarray(gamma, np.float32)
    beta = np.asarray(beta, np.float32)
    w = qkv_w * gamma[:, None]          # fold LN gamma
    bfold = qkv_b + beta @ qkv_w        # fold LN beta
    w3 = w.reshape(DIM, H, 384)
    b3 = bfold.reshape(H, 384)
    # q/k columns, q scaled by SCALE
    wqk = np.concatenate([w3[:, :, :64] * SCALE, w3[:, :, 64:128]], axis=2)
    wqk = wqk.reshape(DIM, H * 128)
    bqk = np.concatenate([b3[:, :64] * SCALE, b3[:, 64:128]], axis=1)
    bqk = bqk.reshape(1, H * 128)
    wv = w3[:, :, 128:].reshape(DIM, DH)
    bv = b3[:, 128:]                    # [H, 256]
    bv1 = np.concatenate(
        [bv, np.ones((H, 1), np.float32), np.zeros((H, 1), np.float32)],
        axis=1,
    ).reshape(1, H * VW)
    bias_full = np.asarray(biases, np.float32)[:, np.asarray(bias_idxs)]
    # device reads bias tiles as [k, q]; transpose (a no-op for the
    # symmetric |dr|,|dc| relative-position bias, but correct in general)
    bias_full = bias_full.transpose(0, 2, 1)
    return {
        "wqk": wqk.astype(ml_dtypes.bfloat16),
        "wv": wv.astype(ml_dtypes.bfloat16),
        "bqk": bqk.astype(ml_dtypes.bfloat16),
        "bv1": bv1.astype(ml_dtypes.bfloat16),
        "pw": np.ascontiguousarray(np.asarray(proj_w, np.float32)).astype(ml_dtypes.bfloat16),
        "pb1": np.asarray(proj_b, np.float32).reshape(1, DIM).astype(ml_dtypes.bfloat16),
        "bias": np.ascontiguousarray(bias_full).astype(ml_dtypes.bfloat16),
        "identb": np.eye(128, dtype=np.float32).astype(ml_dtypes.bfloat16),
        "ones": np.ones((1, 512), ml_dtypes.bfloat16),
    }


def kernel(x, gamma, beta, qkv_w, qkv_b, proj_w, proj_b, biases, bias_idxs,
           _trace=False, _tmpdir=None):
    x = np.asarray(x, np.float32)
    shared = _prep_host(gamma, beta, qkv_w, qkv_b, proj_w, proj_b, biases,
                        bias_idxs)
    flags = (
        bool(np.any(np.asarray(shared["bqk"], np.float32))),
        bool(np.any(np.asarray(shared["bv1"], np.float32)
                    .reshape(H, VW)[:, :256])),
        bool(np.any(np.asarray(shared["pb1"], np.float32))),
    )
    if _CACHE.get("flags") != flags:
        _CACHE["nc"] = build_program(*flags)
        _CACHE["flags"] = flags
    nc = _CACHE["nc"]
    in_maps = []
    for c in range(NCORES):
        m = dict(shared)
        m["x"] = np.ascontiguousarray(x[c * BL:(c + 1) * BL])
        in_maps.append(m)
    res = run_bass_kernel_spmd(
        nc, in_maps, list(range(NCORES)), trace=_trace, tmpdir=_tmpdir,
    )
    _CACHE["last"] = res
    out = np.concatenate([res.results[c]["y"] for c in range(NCORES)], axis=0)
    return out.astype(np.float32)


# revision 39
# speedup vs baseline: 1.2540x; 1.0496x over previous
"""Trainium2 Bass kernel for nn_Attention_51376398794919.

Dense transformer block: LayerNorm -> QKV -> attention with relative-position
bias -> proj.  Data-parallel over batch across 8 NeuronCores (4 batches/core).

Device-side layout strategy (per core):
  - LN in natural layout [tok, d]; xn transposed to xnT [d, tok] via PE
    transposes (stored bf16).
  - qkT (q/k head-transposed, [d_head, tok]) and v-natural ([tok, d_v])
    computed from xnT; q-scale and LN affine folded into weights on host.
  - Scores computed TRANSPOSED: ST[k, q] = kT.T @ qT (bias tiles are read
    [k, q]); the relative-position bias is added by the VECTOR engine
    (tensor_tensor add, in place on the score PSUM tile) instead of burning
    TensorE cycles on identity-matmul bias accumulation; exp on the scalar
    engine PSUM->SBUF (scores are provably < ~4 in magnitude so no
    max-subtraction is needed).
  - PV: out[q, d] = expST.T @ [v | ones | 0]; the ones column yields the
    softmax denominator for free; normalization is a per-partition
    tensor_scalar (258 pad keeps the free size even).
  - attn transposed back (PE) and kept in an SBUF slab; epilogue does proj.
All matmuls run in bf16 with fp32 PSUM accumulation.  PSUM->SBUF copies for
q/k/v run on the scalar engine (activation Copy) to keep the vector engine
free for the bias adds.  The head loop is software-pipelined (qkT/v of head
h+1 emitted between scores(h) and PV(h)).  K=1 bias matmuls are skipped when
the corresponding biases are all zero (checked on the host; build flags
select the general path otherwise).
"""

import sys

import numpy as np

sys.path.insert(0, "/opt/trn_rl_repo")

import concourse.bacc as bacc
import concourse.mybir as mybir
import concourse.tile as tile
from concourse.ap import AP
from concourse.bass_utils import run_bass_kernel_spmd


def strided2(ap, start, stride, n):
    """[128, 2, n] view of a 2D tile AP: two n-wide column blocks at
    `start` and `start+stride` (free-dim elements)."""
    return AP(ap.tensor, ap.offset + start,
              [list(ap.ap[0]), [stride, 2], [1, n]])

# Problem constants
B, N, DIM = 32, 1024, 512
H, KD, D = 8, 64, 256
DH = D * H  # 2048
SCALE = KD ** -0.5
NCORES = 8
BL = B // NCORES  # 4 batches per core

F32 = mybir.dt.float32
F16 = mybir.dt.float16
F8 = mybir.dt.float8e4
BF16 = mybir.dt.bfloat16
AF = mybir.ActivationFunctionType
ALU = mybir.AluOpType

KT = N // 128    # 8 k-tiles
QS = N // 128    # 8 q-slices
DT = DIM // 128  # 4 d-tiles
VW = 258         # v-hat width: 256 v + 1 ones + 1 pad


def build_program(use_qk_bias=False, use_v_bias=False, use_pb=False):
    nc = bacc.Bacc("TRN2", target_bir_lowering=False, debug=True)

    x_d = nc.declare_dram_parameter("x", [BL, N, DIM], F32, isOutput=False)
    wqk_d = nc.declare_dram_parameter("wqk", [DIM, H * 128], BF16, isOutput=False)
    wv_d = nc.declare_dram_parameter("wv", [DIM, DH], BF16, isOutput=False)
    bqk_d = nc.declare_dram_parameter("bqk", [1, H * 128], BF16, isOutput=False)
    bv1_d = nc.declare_dram_parameter("bv1", [1, H * VW], BF16, isOutput=False)
    pw_d = nc.declare_dram_parameter("pw", [DH, DIM], BF16, isOutput=False)
    pb1_d = nc.declare_dram_parameter("pb1", [1, DIM], BF16, isOutput=False)
    bias_d = nc.declare_dram_parameter("bias", [H, N, N], F8, isOutput=False)
    identb_d = nc.declare_dram_parameter("identb", [128, 128], BF16, isOutput=False)
    ones_d = nc.declare_dram_parameter("ones", [1, 512], BF16, isOutput=False)
    y_d = nc.declare_dram_parameter("y", [BL, N, DIM], F32, isOutput=True)

    any_bias = use_qk_bias or use_v_bias or use_pb

    with tile.TileContext(nc) as tc:
        with (
            tc.tile_pool(name="consts", bufs=1) as cpool,
            tc.tile_pool(name="xnt", bufs=2) as xpool,
            tc.tile_pool(name="slab", bufs=1) as slabpool,
            tc.tile_pool(name="yout", bufs=2) as ypool,
            tc.tile_pool(name="ln", bufs=2) as lpool,
            tc.tile_pool(name="xin", bufs=1) as xinpool,
            tc.tile_pool(name="stats", bufs=8) as spool,
            tc.tile_pool(name="sexp", bufs=3) as sxpool,
            tc.tile_pool(name="bias", bufs=4) as bpool,
            tc.tile_pool(name="qk", bufs=2) as qkpool,
            tc.tile_pool(name="vhat", bufs=4) as vpool,
            tc.tile_pool(name="expst", bufs=16) as epool,
            tc.tile_pool(name="attn", bufs=2) as apool,
            tc.tile_pool(name="stp", bufs=2, space="PSUM") as stpp,
            tc.tile_pool(name="pvp", bufs=2, space="PSUM") as pvpp,
            tc.tile_pool(name="miscp", bufs=2, space="PSUM") as mpp,
        ):
            # ---- constants ----
            if any_bias:
                ones_bf = cpool.tile([1, 512], BF16)
                nc.sync.dma_start(ones_bf[:], ones_d[:])
            identb = cpool.tile([128, 128], BF16)
            nc.sync.dma_start(identb[:], identb_d[:])
            eps_t = cpool.tile([128, 1], F32)
            nc.vector.memset(eps_t[:], 1e-5)
            zero_t = cpool.tile([128, 1], F32)
            nc.vector.memset(zero_t[:], 0.0)

            def emit_x_load(b):
                # whole batch of x in two big DMAs (keeps the DMA engines
                # from interleaving 8 small x loads behind weight/bias
                # transfers at startup)
                xf = xinpool.tile([128, QS, DIM], F32, tag="xf")
                src3 = x_d[b].rearrange("(a p) q -> p a q", p=128)
                for g in range(2):
                    nc.gpsimd.dma_start(xf[:, g * 4:(g + 1) * 4, :],
                                        src3[:, g * 4:(g + 1) * 4, :])
                return xf

            xf_cur = emit_x_load(0)
            if use_qk_bias:
                bqk = cpool.tile([1, H * 128], BF16)
                nc.sync.dma_start(bqk[:], bqk_d[:])
            if use_v_bias:
                bv1 = cpool.tile([1, H * VW], BF16)
                nc.sync.dma_start(bv1[:], bv1_d[:])
            # weights: single rearranged DMAs; SP-queue order after identb
            # gives the DMA-engine arrival order x -> identb -> wqk -> wv ->
            # bias(h0); pw is deferred to the pool queue (needed only by the
            # proj epilogue)
            wqk = cpool.tile([128, DT * H * 128], BF16)  # [d-tile][dpart, f]
            nc.sync.dma_start(
                wqk[:].rearrange("p (a q) -> p a q", a=DT),
                wqk_d[:].rearrange("(a p) q -> p a q", p=128),
            )
            wv = cpool.tile([128, DT * DH], BF16)
            if use_pb:
                pb1 = cpool.tile([1, DIM], BF16)
                nc.sync.dma_start(pb1[:], pb1_d[:])
            pw = cpool.tile([128, 16 * DIM], BF16)

            # ---- emitters (batch-parametrized) ----
            def emit_ln_slice(xf, xnt, sl):
                xt = xf[:, sl, :]
                st6 = spool.tile([128, 6], F32, tag="st6")
                nc.vector.bn_stats(st6[:], xt[:])
                mv = spool.tile([128, 2], F32, tag="mv")
                nc.vector.bn_aggr(mv[:], st6[:])
                sd = spool.tile([128, 1], F32, tag="sd")
                nc.scalar.activation(sd[:], mv[:, 1:2], AF.Sqrt, bias=eps_t[:])
                rs = spool.tile([128, 1], F32, tag="rs")
                nc.vector.reciprocal(rs[:], sd[:])
                nm = spool.tile([128, 1], F32, tag="nm")
                nc.vector.tensor_scalar(
                    nm[:], mv[:, 0:1], rs[:], -1.0, ALU.mult, ALU.mult
                )
                xn = lpool.tile([128, DIM], BF16, tag="xn")
                nc.vector.tensor_scalar(
                    xn[:], xt, rs[:], nm[:], ALU.mult, ALU.add
                )
                # 4 transposes into one PSUM tile, one copy out
                tp = mpp.tile([128, 512], BF16, tag="m")
                for dt in range(DT):
                    nc.tensor.transpose(
                        tp[:, dt * 128:(dt + 1) * 128],
                        xn[:, dt * 128:(dt + 1) * 128], identb[:]
                    )
                nc.vector.tensor_copy(
                    xnt[:].rearrange("p (d n) -> p d n", n=N)[:, :, sl * 128:
                                                             (sl + 1) * 128],
                    tp[:].rearrange("p (d n) -> p d n", n=128),
                )

            def emit_bias_dma(h):
                # two [128, 4, 1024] tiles per head (4 k-tiles each),
                # issued from the (otherwise idle) gpsimd SWDGE queue
                src = bias_d[h].rearrange("(a p) q -> p a q", p=128)
                btiles = []
                for g in range(2):
                    bt = bpool.tile([128, 4, N], F8, tag="bias")
                    nc.sync.dma_start(bt[:], src[:, g * 4:(g + 1) * 4, :])
                    btiles.append(bt)
                return btiles

            def alloc_qkv(h):
                # qp rows 0:64 = q (SCALE folded on host), rows 64:128 = k
                qt = qkpool.tile([64, N], BF16, tag="qt")
                ktile = qkpool.tile([64, N], BF16, tag="kt")
                vh = vpool.tile([128, KT * VW], BF16, tag="vh")
                nc.vector.memset(
                    vh[:].rearrange("p (s w) -> p s w", w=VW)[:, :, 256:258],
                    0.0,
                )
                nc.vector.memset(
                    vh[:].rearrange("p (s w) -> p s w", w=VW)[:, :, 256:257],
                    1.0,
                )
                return qt, ktile, vh

            def emit_qk_group(h, qkv, c, xnt):
                qt, ktile, vh = qkv
                qp = mpp.tile([128, 512], F32, tag="m")
                for dt in range(DT):
                    nc.tensor.matmul(
                        qp[:],
                        wqk[:, dt * H * 128 + h * 128:
                            dt * H * 128 + (h + 1) * 128],
                        xnt[:, dt * N + c * 512: dt * N + (c + 1) * 512],
                        start=(dt == 0),
                        stop=(not use_qk_bias and dt == DT - 1),
                    )
                if use_qk_bias:
                    nc.tensor.matmul(
                        qp[:],
                        bqk[:, h * 128:(h + 1) * 128],
                        ones_bf[:, 0:512],
                        start=False,
                        stop=True,
                    )
                nc.scalar.activation(
                    qt[:, c * 512:(c + 1) * 512], qp[0:64, :], AF.Copy
                )
                nc.scalar.activation(
                    ktile[:, c * 512:(c + 1) * 512], qp[64:128, :], AF.Copy
                )

            def emit_v_group(h, qkv, slp, xnt):
                # two tok-slices (2*slp, 2*slp+1) share one PSUM tile and
                # one PSUM->SBUF copy (strided dst into the vh layout)
                qt, ktile, vh = qkv
                vp = mpp.tile([128, 512], F32, tag="m")
                for i in range(2):
                    sl = 2 * slp + i
                    for dt in range(DT):
                        nc.tensor.matmul(
                            vp[:, i * 256:(i + 1) * 256],
                            xnt[:, dt * N + sl * 128: dt * N + (sl + 1) * 128],
                            wv[:, dt * DH + h * 256: dt * DH + (h + 1) * 256],
                            start=(dt == 0),
                            stop=(not use_v_bias and dt == DT - 1),
                        )
                    if use_v_bias:
                        nc.tensor.matmul(
                            vp[:, i * 256:(i + 1) * 256],
                            ones_bf[:, 0:128],
                            bv1[:, h * VW:h * VW + 256],
                            start=False,
                            stop=True,
                            skip_group_check=True,
                        )
                nc.scalar.activation(
                    vh[:].rearrange("p (s w) -> p s w", w=VW)[
                        :, 2 * slp:2 * slp + 2, 0:256],
                    vp[:].rearrange("p (a w) -> p a w", w=256),
                    AF.Copy,
                )

            def emit_score(hctx, et, kt):
                # one [128, 1024] PSUM tile (2 banks): 2 matmuls, then a
                # bias add (vector) writing fp16 SBUF -- this releases the
                # PSUM tile after the ADD (not the exp), giving the score
                # rotation an extra ~1.2us of slack; fp16 is plenty for
                # |scores| < ~4 ahead of the exp
                btiles, (qt, ktile, vh) = hctx
                sp = stpp.tile([128, 1024], F32, tag="st")
                for c in range(2):
                    nc.tensor.matmul(
                        sp[:, c * 512:(c + 1) * 512],
                        ktile[:, kt * 128:(kt + 1) * 128],
                        qt[:, c * 512:(c + 1) * 512],
                        start=True, stop=True,
                    )
                sb16 = sxpool.tile([128, 1024], F16, tag="sb")
                nc.vector.tensor_tensor(
                    sb16[:], sp[:], btiles[kt // 4][:, kt % 4, :], ALU.add
                )
                nc.scalar.activation(et[:], sb16[:], AF.Exp, bias=zero_t[:])

            def emit_pv_slice(h, hctx, est, sl, slab):
                btiles, (qt, ktile, vh) = hctx
                pv = pvpp.tile([128, VW], F32, tag="pv")
                for kt in range(KT):
                    nc.tensor.matmul(
                        pv[:],
                        est[kt][:, sl * 128:(sl + 1) * 128],
                        vh[:, kt * VW:(kt + 1) * VW],
                        start=(kt == 0),
                        stop=(kt == KT - 1),
                    )
                rc = spool.tile([128, 1], F32, tag="rc")
                nc.vector.reciprocal(rc[:], pv[:, 256:257])
                an = apool.tile([128, 256], BF16, tag="an")
                # normalize on the scalar engine (Copy with AP scale); keeps
                # the vector queue short between consecutive bias adds
                nc.scalar.activation(an[:], pv[:, 0:256], AF.Copy,
                                     scale=rc[:])
                tp = mpp.tile([128, 256], BF16, tag="m")
                for dt in range(2):
                    nc.tensor.transpose(
                        tp[:, dt * 128:(dt + 1) * 128],
                        an[:, dt * 128:(dt + 1) * 128], identb[:]
                    )
                nc.vector.tensor_copy(
                    slab[:].rearrange("p (d n) -> p d n", n=N)[
                        :, h * 2:h * 2 + 2, sl * 128:(sl + 1) * 128],
                    tp[:].rearrange("p (a w) -> p a w", w=128),
                )

            def emit_proj_slice(b, slab, sl):
                pp = stpp.tile([128, 512], F32, tag="st")
                for dh in range(16):
                    nc.tensor.matmul(
                        pp[:],
                        slab[:, dh * N + sl * 128: dh * N + (sl + 1) * 128],
                        pw[:, dh * DIM:(dh + 1) * DIM],
                        start=(dh == 0),
                        stop=(not use_pb and dh == 15),
                    )
                if use_pb:
                    nc.tensor.matmul(
                        pp[:], ones_bf[:, 0:128], pb1[:], start=False,
                        stop=True, skip_group_check=True,
                    )
                yt = ypool.tile([128, DIM], F32, tag="y")
                nc.scalar.activation(yt[:], pp[:], AF.Copy)
                nc.gpsimd.dma_start(y_d[b, sl * 128:(sl + 1) * 128, :], yt[:])

            def emit_phase(h, hctx, nxt_qkv, prev, xnt, slab, pre=(),
                           max_slots=99, pre_after_qk=False):
                """Phase h: score groups of head h interleaved with PV
                slices of head h-1 and qkv matmuls of head h+1, so PE
                always has ready work while the vector/scalar engines
                drain the score tiles (bias add + exp)."""
                qkg, vg = [], []
                if nxt_qkv is not None:
                    qkg = [lambda c=c: emit_qk_group(h + 1, nxt_qkv, c, xnt)
                           for c in range(2)]
                    vg = [lambda p=p: emit_v_group(h + 1, nxt_qkv, p, xnt)
                          for p in range(QS // 2)]
                if pre_after_qk:
                    pend = qkg + list(pre) + vg
                else:
                    pend = list(pre) + qkg + vg
                est = []
                gi = 0
                for kt in range(KT):
                    et = epool.tile([128, N], BF16, tag="e")
                    emit_score(hctx, et, kt)
                    if prev is not None:
                        emit_pv_slice(h - 1, prev[0], prev[1], kt, slab)
                    if kt % 4 != 3 and gi < min(len(pend), max_slots):
                        pend[gi]()
                        gi += 1
                    est.append(et)
                while gi < len(pend):
                    pend[gi]()
                    gi += 1
                return est

            # ---- software-pipelined batch loop ----
            # batch 0 prologue: LN, bias DMA, qkv of head 0
            xnt_cur = xpool.tile([128, DT * N], BF16, tag="xnt")
            for sl in range(QS):
                emit_ln_slice(xf_cur, xnt_cur, sl)
            slab_cur = slabpool.tile([128, 16 * N], BF16, tag="slab")
            btiles0 = emit_bias_dma(0)
            nc.sync.dma_start(
                wv[:].rearrange("p (a q) -> p a q", a=DT),
                wv_d[:].rearrange("(a p) q -> p a q", p=128),
            )
            qkv0 = alloc_qkv(0)
            for c in range(2):
                emit_qk_group(0, qkv0, c, xnt_cur)
            hctx = (btiles0, qkv0)
            vpre = [lambda p=p: emit_v_group(0, qkv0, p, xnt_cur)
                    for p in range(QS // 2)]

            for b in range(BL):
                prev = None
                for h in range(H):
                    if h + 1 < H:
                        nb = emit_bias_dma(h + 1)
                        nq = alloc_qkv(h + 1)
                    else:
                        nb, nq = None, None
                    if b == 0 and h == 1:
                        # proj weights: needed only by the epilogue; emitted
                        # here so the transfer queues behind bias h0-h2
                        nc.sync.dma_start(
                            pw[:].rearrange("p (a q) -> p a q", a=16),
                            pw_d[:].rearrange("(a p) q -> p a q", p=128),
                        )
                    est = emit_phase(h, hctx, nq, prev, xnt_cur, slab_cur,
                                     pre=vpre, max_slots=(2 if b == 0 and
                                                          h == 0 else 99),
                                     pre_after_qk=False)
                    vpre = ()
                    prev = (hctx, est)
                    hctx = (nb, nq) if nq is not None else None
                # fused tail: drain PV of head 7 + proj(b) + LN(b+1),
                # interleaved to keep all engines fed across the batch edge
                if b + 1 < BL:
                    xnt_next = xpool.tile([128, DT * N], BF16, tag="xnt")
                    xf_cur = emit_x_load(b + 1)
                else:
                    xnt_next = None
                ndefer = 0
                for sl in range(QS):
                    emit_pv_slice(H - 1, prev[0], prev[1], sl, slab_cur)
                    if 0 < sl < QS - ndefer:
                        emit_proj_slice(b, slab_cur, sl - 1)
                    if xnt_next is not None:
                        emit_ln_slice(xf_cur, xnt_next, sl)
                if ndefer == 0:
                    emit_proj_slice(b, slab_cur, QS - 1)
                else:
                    # last proj slices ride in the next batch's phase-0
                    # interleave (phase 0 has no PV work and starves PE)
                    dproj = [lambda s=s, bb=b, sc=slab_cur:
                             emit_proj_slice(bb, sc, s)
                             for s in range(QS - 1 - ndefer, QS)]
                if b + 1 < BL:
                    xnt_cur = xnt_next
                    slab_cur = slabpool.tile([128, 16 * N], BF16, tag="slab")
                    btiles0 = emit_bias_dma(0)
                    qkv0 = alloc_qkv(0)
                    for c in range(2):
                        emit_qk_group(0, qkv0, c, xnt_cur)
                    hctx = (btiles0, qkv0)
                    vpre = [lambda p=p, q=qkv0, x=xnt_cur:
                            emit_v_group(0, q, p, x)
                            for p in range(QS // 2)]


    nc.compile()
    return nc


_CACHE = {}


def _prep_host(gamma, beta, qkv_w, qkv_b, proj_w, proj_b, biases, bias_idxs):
    import ml_dtypes

    qkv_w = np.asarray(qkv_w, np.float32)
    qkv_b = np.asarray(qkv_b, np.float32)
    gamma = np.asarray(gamma, np.float32)
    beta = np.asarray(beta, np.float32)
    w = qkv_w * gamma[:, None]          # fold LN gamma
    bfold = qkv_b + beta @ qkv_w        # fold LN beta
    w3 = w.reshape(DIM, H, 384)
    b3 = bfold.reshape(H, 384)
    # q/k columns, q scaled by SCALE
    wqk = np.concatenate([w3[:, :, :64] * SCALE, w3[:, :, 64:128]], axis=2)
    wqk = wqk.reshape(DIM, H * 128)
    bqk = np.concatenate([b3[:, :64] * SCALE, b3[:, 64:128]], axis=1)
    bqk = bqk.reshape(1, H * 128)
    wv = w3[:, :, 128:].reshape(DIM, DH)
    bv = b3[:, 128:]                    # [H, 256]
    bv1 = np.concatenate(
        [bv, np.ones((H, 1), np.float32), np.zeros((H, 1), np.float32)],
        axis=1,
    ).reshape(1, H * VW)
    bias_full = np.asarray(biases, np.float32)[:, np.asarray(bias_idxs)]
    # device reads bias tiles as [k, q]; transpose (a no-op for the
    # symmetric |dr|,|dc| relative-position bias, but correct in general)
    bias_full = bias_full.transpose(0, 2, 1)
    return {
        "wqk": wqk.astype(ml_dtypes.bfloat16),
        "wv": wv.astype(ml_dtypes.bfloat16),
        "bqk": bqk.astype(ml_dtypes.bfloat16),
        "bv1": bv1.astype(ml_dtypes.bfloat16),
        "pw": np.ascontiguousarray(np.asarray(proj_w, np.float32)).astype(ml_dtypes.bfloat16),
        "pb1": np.asarray(proj_b, np.float32).reshape(1, DIM).astype(ml_dtypes.bfloat16),
        "bias": np.ascontiguousarray(bias_full).astype(ml_dtypes.float8_e4m3),
        "identb": np.eye(128, dtype=np.float32).astype(ml_dtypes.bfloat16),
        "ones": np.ones((1, 512), ml_dtypes.bfloat16),
    }


def kernel(x, gamma, beta, qkv_w, qkv_b, proj_w, proj_b, biases, bias_idxs,
           _trace=False, _tmpdir=None):
    x = np.asarray(x, np.float32)
    shared = _prep_host(gamma, beta, qkv_w, qkv_b, proj_w, proj_b, biases,
                        bias_idxs)
    flags = (
        bool(np.any(np.asarray(shared["bqk"], np.float32))),
        bool(np.any(np.asarray(shared["bv1"], np.float32)
                    .reshape(H, VW)[:, :256])),
        bool(np.any(np.asarray(shared["pb1"], np.float32))),
    )
    if _CACHE.get("flags") != flags:
        _CACHE["nc"] = build_program(*flags)
        _CACHE["flags"] = flags
    nc = _CACHE["nc"]
    in_maps = []
    for c in range(NCORES):
        m = dict(shared)
        m["x"] = np.ascontiguousarray(x[c * BL:(c + 1) * BL])
        in_maps.append(m)
    res = run_bass_kernel_spmd(
        nc, in_maps, list(range(NCORES)), trace=_trace, tmpdir=_tmpdir,
    )
    _CACHE["last"] = res
    out = np.concatenate([res.results[c]["y"] for c in range(NCORES)], axis=0)
    return out.astype(np.float32)


# revision 44
# speedup vs baseline: 1.2621x; 1.0065x over previous
"""Trainium2 Bass kernel for nn_Attention_51376398794919.

Dense transformer block: LayerNorm -> QKV -> attention with relative-position
bias -> proj.  Data-parallel over batch across 8 NeuronCores (4 batches/core).

Device-side layout strategy (per core):
  - LN in natural layout [tok, d]; xn transposed to xnT [d, tok] via PE
    transposes (stored bf16).
  - qkT (q/k head-transposed, [d_head, tok]) and v-natural ([tok, d_v])
    computed from xnT; q-scale and LN affine folded into weights on host.
  - Scores computed TRANSPOSED: ST[k, q] = kT.T @ qT (bias tiles are read
    [k, q]); the relative-position bias is added by the VECTOR engine
    (tensor_tensor add, in place on the score PSUM tile) instead of burning
    TensorE cycles on identity-matmul bias accumulation; exp on the scalar
    engine PSUM->SBUF (scores are provably < ~4 in magnitude so no
    max-subtraction is needed).
  - PV: out[q, d] = expST.T @ [v | ones | 0]; the ones column yields the
    softmax denominator for free; normalization is a per-partition
    tensor_scalar (258 pad keeps the free size even).
  - attn transposed back (PE) and kept in an SBUF slab; epilogue does proj.
All matmuls run in bf16 with fp32 PSUM accumulation.  PSUM->SBUF copies for
q/k/v run on the scalar engine (activation Copy) to keep the vector engine
free for the bias adds.  The head loop is software-pipelined (qkT/v of head
h+1 emitted between scores(h) and PV(h)).  K=1 bias matmuls are skipped when
the corresponding biases are all zero (checked on the host; build flags
select the general path otherwise).
"""

import sys

import numpy as np

sys.path.insert(0, "/opt/trn_rl_repo")

import concourse.bacc as bacc
import concourse.mybir as mybir
import concourse.tile as tile
from concourse.ap import AP
from concourse.bass_utils import run_bass_kernel_spmd


def strided2(ap, start, stride, n):
    """[128, 2, n] view of a 2D tile AP: two n-wide column blocks at
    `start` and `start+stride` (free-dim elements)."""
    return AP(ap.tensor, ap.offset + start,
              [list(ap.ap[0]), [stride, 2], [1, n]])

# Problem constants
B, N, DIM = 32, 1024, 512
H, KD, D = 8, 64, 256
DH = D * H  # 2048
SCALE = KD ** -0.5
NCORES = 8
BL = B // NCORES  # 4 batches per core

F32 = mybir.dt.float32
F16 = mybir.dt.float16
F8 = mybir.dt.float8e4
BF16 = mybir.dt.bfloat16
AF = mybir.ActivationFunctionType
ALU = mybir.AluOpType

KT = N // 128    # 8 k-tiles
QS = N // 128    # 8 q-slices
DT = DIM // 128  # 4 d-tiles
VW = 258         # v-hat width: 256 v + 1 ones + 1 pad


def build_program(use_qk_bias=False, use_v_bias=False, use_pb=False):
    nc = bacc.Bacc("TRN2", target_bir_lowering=False, debug=True)

    x_d = nc.declare_dram_parameter("x", [BL, N, DIM], F32, isOutput=False)
    wqk_d = nc.declare_dram_parameter("wqk", [DIM, H * 128], BF16, isOutput=False)
    wv_d = nc.declare_dram_parameter("wv", [DIM, DH], BF16, isOutput=False)
    bqk_d = nc.declare_dram_parameter("bqk", [1, H * 128], BF16, isOutput=False)
    bv1_d = nc.declare_dram_parameter("bv1", [1, H * VW], BF16, isOutput=False)
    pw_d = nc.declare_dram_parameter("pw", [DH, DIM], BF16, isOutput=False)
    pb1_d = nc.declare_dram_parameter("pb1", [1, DIM], BF16, isOutput=False)
    bias_d = nc.declare_dram_parameter("bias", [H, N, N], F8, isOutput=False)
    identb_d = nc.declare_dram_parameter("identb", [128, 128], BF16, isOutput=False)
    ones_d = nc.declare_dram_parameter("ones", [1, 512], BF16, isOutput=False)
    y_d = nc.declare_dram_parameter("y", [BL, N, DIM], F32, isOutput=True)

    any_bias = use_qk_bias or use_v_bias or use_pb

    with tile.TileContext(nc) as tc:
        with (
            tc.tile_pool(name="consts", bufs=1) as cpool,
            tc.tile_pool(name="xnt", bufs=2) as xpool,
            tc.tile_pool(name="slab", bufs=1) as slabpool,
            tc.tile_pool(name="yout", bufs=2) as ypool,
            tc.tile_pool(name="ln", bufs=2) as lpool,
            tc.tile_pool(name="xin", bufs=1) as xinpool,
            tc.tile_pool(name="stats", bufs=8) as spool,
            tc.tile_pool(name="sexp", bufs=3) as sxpool,
            tc.tile_pool(name="bias", bufs=2) as bpool,
            tc.tile_pool(name="qk", bufs=2) as qkpool,
            tc.tile_pool(name="vhat", bufs=4) as vpool,
            tc.tile_pool(name="expst", bufs=16) as epool,
            tc.tile_pool(name="attn", bufs=2) as apool,
            tc.tile_pool(name="stp", bufs=2, space="PSUM") as stpp,
            tc.tile_pool(name="pvp", bufs=2, space="PSUM") as pvpp,
            tc.tile_pool(name="miscp", bufs=2, space="PSUM") as mpp,
        ):
            # ---- constants ----
            if any_bias:
                ones_bf = cpool.tile([1, 512], BF16)
                nc.sync.dma_start(ones_bf[:], ones_d[:])
            identb = cpool.tile([128, 128], BF16)
            nc.sync.dma_start(identb[:], identb_d[:])
            eps_t = cpool.tile([128, 1], F32)
            nc.vector.memset(eps_t[:], 1e-5)
            zero_t = cpool.tile([128, 1], F32)
            nc.vector.memset(zero_t[:], 0.0)

            def emit_x_load(b):
                # whole batch of x in two big DMAs (keeps the DMA engines
                # from interleaving 8 small x loads behind weight/bias
                # transfers at startup)
                xf = xinpool.tile([128, QS, DIM], F32, tag="xf")
                src3 = x_d[b].rearrange("(a p) q -> p a q", p=128)
                for g in range(2):
                    nc.gpsimd.dma_start(xf[:, g * 4:(g + 1) * 4, :],
                                        src3[:, g * 4:(g + 1) * 4, :])
                return xf

            xf_cur = emit_x_load(0)
            if use_qk_bias:
                bqk = cpool.tile([1, H * 128], BF16)
                nc.sync.dma_start(bqk[:], bqk_d[:])
            if use_v_bias:
                bv1 = cpool.tile([1, H * VW], BF16)
                nc.sync.dma_start(bv1[:], bv1_d[:])
            # weights: single rearranged DMAs; SP-queue order after identb
            # gives the DMA-engine arrival order x -> identb -> wqk -> wv ->
            # bias(h0); pw is deferred to the pool queue (needed only by the
            # proj epilogue)
            wqk = cpool.tile([128, DT * H * 128], BF16)  # [d-tile][dpart, f]
            nc.sync.dma_start(
                wqk[:].rearrange("p (a q) -> p a q", a=DT),
                wqk_d[:].rearrange("(a p) q -> p a q", p=128),
            )
            wv = cpool.tile([128, DT * DH], BF16)
            if use_pb:
                pb1 = cpool.tile([1, DIM], BF16)
                nc.sync.dma_start(pb1[:], pb1_d[:])
            pw = cpool.tile([128, 16 * DIM], BF16)

            # ---- emitters (batch-parametrized) ----
            def emit_ln_slice(xf, xnt, sl):
                xt = xf[:, sl, :]
                st6 = spool.tile([128, 6], F32, tag="st6")
                nc.vector.bn_stats(st6[:], xt[:])
                mv = spool.tile([128, 2], F32, tag="mv")
                nc.vector.bn_aggr(mv[:], st6[:])
                sd = spool.tile([128, 1], F32, tag="sd")
                nc.scalar.activation(sd[:], mv[:, 1:2], AF.Sqrt, bias=eps_t[:])
                rs = spool.tile([128, 1], F32, tag="rs")
                nc.vector.reciprocal(rs[:], sd[:])
                nm = spool.tile([128, 1], F32, tag="nm")
                nc.vector.tensor_scalar(
                    nm[:], mv[:, 0:1], rs[:], -1.0, ALU.mult, ALU.mult
                )
                xn = lpool.tile([128, DIM], BF16, tag="xn")
                nc.vector.tensor_scalar(
                    xn[:], xt, rs[:], nm[:], ALU.mult, ALU.add
                )
                # 4 transposes into one PSUM tile, one copy out
                tp = mpp.tile([128, 512], BF16, tag="m")
                for dt in range(DT):
                    nc.tensor.transpose(
                        tp[:, dt * 128:(dt + 1) * 128],
                        xn[:, dt * 128:(dt + 1) * 128], identb[:]
                    )
                nc.vector.tensor_copy(
                    xnt[:].rearrange("p (d n) -> p d n", n=N)[:, :, sl * 128:
                                                             (sl + 1) * 128],
                    tp[:].rearrange("p (d n) -> p d n", n=128),
                )

            def emit_bias_dma(h):
                # one [128, 8, 1024] fp8 tile per head, filled by two DMAs
                # (first half arrives earlier and un-gates the first adds)
                src = bias_d[h].rearrange("(a p) q -> p a q", p=128)
                bt = bpool.tile([128, KT, N], F8, tag="bias")
                for g in range(2):
                    nc.sync.dma_start(bt[:, g * 4:(g + 1) * 4, :],
                                      src[:, g * 4:(g + 1) * 4, :])
                return bt

            def alloc_qkv(h):
                # qp rows 0:64 = q (SCALE folded on host), rows 64:128 = k
                qt = qkpool.tile([64, N], BF16, tag="qt")
                ktile = qkpool.tile([64, N], BF16, tag="kt")
                vh = vpool.tile([128, KT * VW], BF16, tag="vh")
                nc.vector.memset(
                    vh[:].rearrange("p (s w) -> p s w", w=VW)[:, :, 256:258],
                    0.0,
                )
                nc.vector.memset(
                    vh[:].rearrange("p (s w) -> p s w", w=VW)[:, :, 256:257],
                    1.0,
                )
                return qt, ktile, vh

            def emit_qk_group(h, qkv, c, xnt):
                qt, ktile, vh = qkv
                qp = mpp.tile([128, 512], F32, tag="m")
                for dt in range(DT):
                    nc.tensor.matmul(
                        qp[:],
                        wqk[:, dt * H * 128 + h * 128:
                            dt * H * 128 + (h + 1) * 128],
                        xnt[:, dt * N + c * 512: dt * N + (c + 1) * 512],
                        start=(dt == 0),
                        stop=(not use_qk_bias and dt == DT - 1),
                    )
                if use_qk_bias:
                    nc.tensor.matmul(
                        qp[:],
                        bqk[:, h * 128:(h + 1) * 128],
                        ones_bf[:, 0:512],
                        start=False,
                        stop=True,
                    )
                nc.scalar.activation(
                    qt[:, c * 512:(c + 1) * 512], qp[0:64, :], AF.Copy
                )
                nc.scalar.activation(
                    ktile[:, c * 512:(c + 1) * 512], qp[64:128, :], AF.Copy
                )

            def emit_v_group(h, qkv, slp, xnt):
                # two tok-slices (2*slp, 2*slp+1) share one PSUM tile and
                # one PSUM->SBUF copy (strided dst into the vh layout)
                qt, ktile, vh = qkv
                vp = mpp.tile([128, 512], F32, tag="m")
                for i in range(2):
                    sl = 2 * slp + i
                    for dt in range(DT):
                        nc.tensor.matmul(
                            vp[:, i * 256:(i + 1) * 256],
                            xnt[:, dt * N + sl * 128: dt * N + (sl + 1) * 128],
                            wv[:, dt * DH + h * 256: dt * DH + (h + 1) * 256],
                            start=(dt == 0),
                            stop=(not use_v_bias and dt == DT - 1),
                        )
                    if use_v_bias:
                        nc.tensor.matmul(
                            vp[:, i * 256:(i + 1) * 256],
                            ones_bf[:, 0:128],
                            bv1[:, h * VW:h * VW + 256],
                            start=False,
                            stop=True,
                            skip_group_check=True,
                        )
                nc.vector.tensor_copy(
                    vh[:].rearrange("p (s w) -> p s w", w=VW)[
                        :, 2 * slp:2 * slp + 2, 0:256],
                    vp[:].rearrange("p (a w) -> p a w", w=256),
                )

            def emit_score(hctx, et, kt):
                # one [128, 1024] PSUM tile (2 banks): 2 matmuls, then a
                # bias add (vector) writing fp16 SBUF -- this releases the
                # PSUM tile after the ADD (not the exp), giving the score
                # rotation an extra ~1.2us of slack; fp16 is plenty for
                # |scores| < ~4 ahead of the exp
                btiles, (qt, ktile, vh) = hctx
                sp = stpp.tile([128, 1024], F32, tag="st")
                for c in range(2):
                    nc.tensor.matmul(
                        sp[:, c * 512:(c + 1) * 512],
                        ktile[:, kt * 128:(kt + 1) * 128],
                        qt[:, c * 512:(c + 1) * 512],
                        start=True, stop=True,
                    )
                sb16 = sxpool.tile([128, 1024], F16, tag="sb")
                nc.vector.tensor_tensor(
                    sb16[:], sp[:], btiles[:, kt, :], ALU.add
                )
                nc.scalar.activation(et[:], sb16[:], AF.Exp, bias=zero_t[:])

            def emit_pv_slice(h, hctx, est, sl, slab):
                btiles, (qt, ktile, vh) = hctx
                pv = pvpp.tile([128, VW], F32, tag="pv")
                for kt in range(KT):
                    nc.tensor.matmul(
                        pv[:],
                        est[kt][:, sl * 128:(sl + 1) * 128],
                        vh[:, kt * VW:(kt + 1) * VW],
                        start=(kt == 0),
                        stop=(kt == KT - 1),
                    )
                rc = spool.tile([128, 1], F32, tag="rc")
                nc.vector.reciprocal(rc[:], pv[:, 256:257])
                an = apool.tile([128, 256], BF16, tag="an")
                # normalize on the scalar engine (Copy with AP scale); keeps
                # the vector queue short between consecutive bias adds
                nc.scalar.activation(an[:], pv[:, 0:256], AF.Copy,
                                     scale=rc[:])
                tp = mpp.tile([128, 256], BF16, tag="m")
                for dt in range(2):
                    nc.tensor.transpose(
                        tp[:, dt * 128:(dt + 1) * 128],
                        an[:, dt * 128:(dt + 1) * 128], identb[:]
                    )
                nc.vector.tensor_copy(
                    slab[:].rearrange("p (d n) -> p d n", n=N)[
                        :, h * 2:h * 2 + 2, sl * 128:(sl + 1) * 128],
                    tp[:].rearrange("p (a w) -> p a w", w=128),
                )

            def emit_proj_slice(b, slab, sl):
                pp = stpp.tile([128, 512], F32, tag="st")
                for dh in range(16):
                    nc.tensor.matmul(
                        pp[:],
                        slab[:, dh * N + sl * 128: dh * N + (sl + 1) * 128],
                        pw[:, dh * DIM:(dh + 1) * DIM],
                        start=(dh == 0),
                        stop=(not use_pb and dh == 15),
                    )
                if use_pb:
                    nc.tensor.matmul(
                        pp[:], ones_bf[:, 0:128], pb1[:], start=False,
                        stop=True, skip_group_check=True,
                    )
                yt = ypool.tile([128, DIM], F32, tag="y")
                nc.scalar.activation(yt[:], pp[:], AF.Copy)
                nc.gpsimd.dma_start(y_d[b, sl * 128:(sl + 1) * 128, :], yt[:])

            def emit_phase(h, hctx, nxt_qkv, prev, xnt, slab, pre=(),
                           max_slots=99, pre_after_qk=False):
                """Phase h: score groups of head h interleaved with PV
                slices of head h-1 and qkv matmuls of head h+1, so PE
                always has ready work while the vector/scalar engines
                drain the score tiles (bias add + exp)."""
                qkg, vg = [], []
                if nxt_qkv is not None:
                    qkg = [lambda c=c: emit_qk_group(h + 1, nxt_qkv, c, xnt)
                           for c in range(2)]
                    vg = [lambda p=p: emit_v_group(h + 1, nxt_qkv, p, xnt)
                          for p in range(QS // 2)]
                if pre_after_qk:
                    pend = qkg + list(pre) + vg
                else:
                    pend = list(pre) + qkg + vg
                est = []
                gi = 0
                for kt in range(KT):
                    et = epool.tile([128, N], BF16, tag="e")
                    emit_score(hctx, et, kt)
                    if prev is not None:
                        emit_pv_slice(h - 1, prev[0], prev[1], kt, slab)
                    if kt % 4 != 3 and gi < min(len(pend), max_slots):
                        pend[gi]()
                        gi += 1
                    est.append(et)
                while gi < len(pend):
                    pend[gi]()
                    gi += 1
                return est

            # ---- software-pipelined batch loop ----
            # batch 0 prologue: LN, bias DMA, qkv of head 0
            xnt_cur = xpool.tile([128, DT * N], BF16, tag="xnt")
            for sl in range(QS):
                emit_ln_slice(xf_cur, xnt_cur, sl)
            slab_cur = slabpool.tile([128, 16 * N], BF16, tag="slab")
            btiles0 = emit_bias_dma(0)
            nc.sync.dma_start(
                wv[:].rearrange("p (a q) -> p a q", a=DT),
                wv_d[:].rearrange("(a p) q -> p a q", p=128),
            )
            qkv0 = alloc_qkv(0)
            for c in range(2):
                emit_qk_group(0, qkv0, c, xnt_cur)
            hctx = (btiles0, qkv0)
            vpre = [lambda p=p: emit_v_group(0, qkv0, p, xnt_cur)
                    for p in range(QS // 2)]

            for b in range(BL):
                prev = None
                for h in range(H):
                    if h + 1 < H:
                        nb = emit_bias_dma(h + 1)
                        nq = alloc_qkv(h + 1)
                    else:
                        nb, nq = None, None
                    if b == 0 and h == 1:
                        # proj weights: needed only by the epilogue; emitted
                        # here so the transfer queues behind bias h0-h2
                        nc.sync.dma_start(
                            pw[:].rearrange("p (a q) -> p a q", a=16),
                            pw_d[:].rearrange("(a p) q -> p a q", p=128),
                        )
                    est = emit_phase(h, hctx, nq, prev, xnt_cur, slab_cur,
                                     pre=vpre, max_slots=(2 if b == 0 and
                                                          h == 0 else 99),
                                     pre_after_qk=False)
                    vpre = ()
                    prev = (hctx, est)
                    hctx = (nb, nq) if nq is not None else None
                # fused tail: drain PV of head 7 + proj(b) + LN(b+1),
                # interleaved to keep all engines fed across the batch edge
                if b + 1 < BL:
                    xnt_next = xpool.tile([128, DT * N], BF16, tag="xnt")
                    xf_cur = emit_x_load(b + 1)
                else:
                    xnt_next = None
                ndefer = 0
                for sl in range(QS):
                    emit_pv_slice(H - 1, prev[0], prev[1], sl, slab_cur)
                    if 0 < sl < QS - ndefer:
                        emit_proj_slice(b, slab_cur, sl - 1)
                    if xnt_next is not None:
                        emit_ln_slice(xf_cur, xnt_next, sl)
                if ndefer == 0:
                    emit_proj_slice(b, slab_cur, QS - 1)
                else:
                    # last proj slices ride in the next batch's phase-0
                    # interleave (phase 0 has no PV work and starves PE)
                    dproj = [lambda s=s, bb=b, sc=slab_cur:
                             emit_proj_slice(bb, sc, s)
                             for s in range(QS - 1 - ndefer, QS)]
                if b + 1 < BL:
                    xnt_cur = xnt_next
                    slab_cur = slabpool.tile([128, 16 * N], BF16, tag="slab")
                    btiles0 = emit_bias_dma(0)
                    qkv0 = alloc_qkv(0)
                    for c in range(2):
                        emit_qk_group(0, qkv0, c, xnt_cur)
                    hctx = (btiles0, qkv0)
                    vpre = [lambda p=p, q=qkv0, x=xnt_cur:
                            emit_v_group(0, q, p, x)
                            for p in range(QS // 2)]


    nc.compile()
    return nc


_CACHE = {}


def _prep_host(gamma, beta, qkv_w, qkv_b, proj_w, proj_b, biases, bias_idxs):
    import ml_dtypes

    qkv_w = np.asarray(qkv_w, np.float32)
    qkv_b = np.asarray(qkv_b, np.float32)
    gamma = np.asarray(gamma, np.float32)
    beta = np.asarray(beta, np.float32)
    w = qkv_w * gamma[:, None]          # fold LN gamma
    bfold = qkv_b + beta @ qkv_w        # fold LN beta
    w3 = w.reshape(DIM, H, 384)
    b3 = bfold.reshape(H, 384)
    # q/k columns, q scaled by SCALE
    wqk = np.concatenate([w3[:, :, :64] * SCALE, w3[:, :, 64:128]], axis=2)
    wqk = wqk.reshape(DIM, H * 128)
    bqk = np.concatenate([b3[:, :64] * SCALE, b3[:, 64:128]], axis=1)
    bqk = bqk.reshape(1, H * 128)
    wv = w3[:, :, 128:].reshape(DIM, DH)
    bv = b3[:, 128:]                    # [H, 256]
    bv1 = np.concatenate(
        [bv, np.ones((H, 1), np.float32), np.zeros((H, 1), np.float32)],
        axis=1,
    ).reshape(1, H * VW)
    bias_full = np.asarray(biases, np.float32)[:, np.asarray(bias_idxs)]
    # device reads bias tiles as [k, q]; transpose (a no-op for the
    # symmetric |dr|,|dc| relative-position bias, but correct in general)
    bias_full = bias_full.transpose(0, 2, 1)
    return {
        "wqk": wqk.astype(ml_dtypes.bfloat16),
        "wv": wv.astype(ml_dtypes.bfloat16),
        "bqk": bqk.astype(ml_dtypes.bfloat16),
        "bv1": bv1.astype(ml_dtypes.bfloat16),
        "pw": np.ascontiguousarray(np.asarray(proj_w, np.float32)).astype(ml_dtypes.bfloat16),
        "pb1": np.asarray(proj_b, np.float32).reshape(1, DIM).astype(ml_dtypes.bfloat16),
        "bias": np.ascontiguousarray(bias_full).astype(ml_dtypes.float8_e4m3),
        "identb": np.eye(128, dtype=np.float32).astype(ml_dtypes.bfloat16),
        "ones": np.ones((1, 512), ml_dtypes.bfloat16),
    }


def kernel(x, gamma, beta, qkv_w, qkv_b, proj_w, proj_b, biases, bias_idxs,
           _trace=False, _tmpdir=None):
    x = np.asarray(x, np.float32)
    shared = _prep_host(gamma, beta, qkv_w, qkv_b, proj_w, proj_b, biases,
                        bias_idxs)
    flags = (
        bool(np.any(np.asarray(shared["bqk"], np.float32))),
        bool(np.any(np.asarray(shared["bv1"], np.float32)
                    .reshape(H, VW)[:, :256])),
        bool(np.any(np.asarray(shared["pb1"], np.float32))),
    )
    if _CACHE.get("flags") != flags:
        _CACHE["nc"] = build_program(*flags)
        _CACHE["flags"] = flags
    nc = _CACHE["nc"]
    in_maps = []
    for c in range(NCORES):
        m = dict(shared)
        m["x"] = np.ascontiguousarray(x[c * BL:(c + 1) * BL])
        in_maps.append(m)
    res = run_bass_kernel_spmd(
        nc, in_maps, list(range(NCORES)), trace=_trace, tmpdir=_tmpdir,
    )
    _CACHE["last"] = res
    out = np.concatenate([res.results[c]["y"] for c in range(NCORES)], axis=0)
    return out.astype(np.float32)
